# revision 43
# baseline (speedup 1.0000x reference)
"""kNN (k=16) + grouped 3->64->64->64 MLP + neighbor max-pool on 8 TRN2 cores.

Pipeline (device does all O(N^2) compute, selection, and MLP flops):
  L1 : S[q,j] = 2<xq,xj> - |xj|^2 on PE via a triple-bf16-split 21-contract
       matmul (1 cyc/row, ~1e-6 exact); chunk-16 max via a DVE fold tree with
       the 8->4 and 2->1 levels done as relu-max (pool sub + ACT relu + pool
       add); top-24 chunk ids per query via 3 rounds of max8/max_index/
       match_replace (host uses the first 20; self chunk is always rank 1,
       so that is self + top-19 -- the guarantee needs 17, margin measured).
  host: gather 20 chunks = 320 candidate coords per query, replace the self
       point with a far dummy (index routing only).
  L2A: exact squared dists in reference fp32 arithmetic on the 320-wide
       compacted domain; unordered exact top-16 (2 rounds) -> local indices.
  host: map local->global indices, gather the 16 neighbor coords.
  L2B: relative coords via matmul-folded subtract (single 12-contract fp32r
       matmul), packed 2-point 3-layer MLP on PE (fp32r, tol 2e-2 absorbs
       the ~2e-4 rounding), max-pool over neighbors, PE-transpose, final max.

Sharding: core c handles batch c//2, query half c%2 (2048 queries each).
"""
import sys
import numpy as np

sys.path.insert(0, "/opt/trn_rl_repo")

import jax
import numpy as _np
import ml_dtypes
from jax.sharding import Mesh, PartitionSpec
from jax.experimental.shard_map import shard_map

import concourse.bacc as bacc
import concourse.mybir as mybir
import concourse.tile as tile
from concourse import bass2jax
from concourse.bass2jax import _bass_exec_p, install_neuronx_cc_hook

F32 = mybir.dt.float32
F32R = mybir.dt.float32r
BF16 = mybir.dt.bfloat16
U16 = mybir.dt.uint16
AX = mybir.AxisListType
OP = mybir.AluOpType
AF = mybir.ActivationFunctionType
BF = ml_dtypes.bfloat16

B, N, C, K = 4, 4096, 64, 16
CH = 16                 # chunk size for the selection hierarchy
NCH = N // CH           # 256
NIDS = 24               # ids written per query (3 rounds of 8)
NSEL = 20               # chunks used per query (self rank-1 + top-19)
W = NSEL * CH           # 320 candidate superset per query
NQ = 2048               # queries per core
NBLK = NQ // 128        # 16
NROW = 21               # triple-split contraction rows
NEG = -1.0e30
NCORES = 8

_progs = {}


def _rounds(nc, sp, vals, out_ids, tag, nr):
    """nr x (max8 -> max_index -> match_replace): top-8*nr ids, descending."""
    for r in range(nr):
        m8 = sp.tile([128, 8], F32, tag=f"m8{tag}", name=f"m8{tag}_{r}_{id(vals)}")
        nc.vector.max(out=m8[:], in_=vals)
        nc.vector.max_index(out=out_ids[:, r * 8:(r + 1) * 8], in_max=m8[:],
                            in_values=vals)
        if r < nr - 1:
            nc.vector.match_replace(out=vals, in_to_replace=m8[:], in_values=vals,
                                    imm_value=NEG)


def _relu_max(nc, wp, out, a, b, width, tag, i):
    """out = max(a, b) elementwise as relu(a-b)+b: pool sub, ACT relu, pool
    add. Frees the DVE; exact to ~1ulp (covered by the selection margin)."""
    d = wp.tile([128, width], F32, tag=f"d{tag}", name=f"d{tag}_{i}")
    nc.gpsimd.tensor_tensor(d[:], a, b, op=OP.subtract)
    r = wp.tile([128, width], F32, tag=f"r{tag}", name=f"r{tag}_{i}")
    nc.scalar.activation(r[:], d[:], AF.Relu)
    nc.gpsimd.tensor_tensor(out, r[:], b, op=OP.add)


def _build_l1(repeat=1):
    nc = bacc.Bacc("TRN2", target_bir_lowering=False, debug=False,
                   num_devices=NCORES)
    xyzS_d = nc.dram_tensor("xyzS", [NROW, N], BF16, kind="ExternalInput").ap()
    qS_d = nc.dram_tensor("qS", [NROW, NQ], BF16, kind="ExternalInput").ap()
    ids_d = nc.dram_tensor("ids", [NQ, NIDS], U16, kind="ExternalOutput").ap()
    with tile.TileContext(nc) as tc:
        with (
            tc.tile_pool(name="tabs", bufs=1) as tabs,
            tc.tile_pool(name="psum", bufs=2, space="PSUM") as pp,
            tc.tile_pool(name="work", bufs=4) as wp,
            tc.tile_pool(name="small", bufs=8) as sp,
        ):
            xyzS_sb = tabs.tile([NROW, N], BF16)
            qS_sb = tabs.tile([NROW, NQ], BF16)
            nc.sync.dma_start(out=qS_sb[:], in_=qS_d[:])
            for dj in range(4):
                nc.sync.dma_start(out=xyzS_sb[:, dj * 1024:(dj + 1) * 1024],
                                  in_=xyzS_d[:, dj * 1024:(dj + 1) * 1024])
            for i in range(repeat * NBLK):
                ib = i % NBLK
                lhsT = qS_sb[:, ib * 128:(ib + 1) * 128]
                f8 = wp.tile([128, NCH, 8], F32, tag="f8", name=f"f8_{i}")
                for t in range(2):
                    ps = pp.tile([128, 2048], F32, tag="ps", name=f"ps_{i}_{t}")
                    for n in range(4):
                        j = t * 4 + n
                        nc.tensor.matmul(ps[:, n * 512:(n + 1) * 512], lhsT,
                                         xyzS_sb[:, j * 512:(j + 1) * 512],
                                         start=True, stop=True)
                    # fold 16 -> 8: ACT stages the odd half in SBUF (only one
                    # PSUM operand allowed per instruction), DVE TT-maxes it
                    # against the even half still in PSUM
                    ps3d = ps[:].rearrange("p (c w) -> p c w", w=CH)
                    so = wp.tile([128, 128, 8], F32, tag="so",
                                 name=f"so_{i}_{t}")
                    nc.scalar.activation(so[:], ps3d[:, :, 8:16], AF.Copy)
                    nc.vector.tensor_tensor(
                        f8[:, t * 128:(t + 1) * 128, :],
                        ps3d[:, :, 0:8], so[:], op=OP.max)
                # fold 8 -> 4 on pool+ACT (relu-max), 4 -> 2 on DVE,
                # 2 -> 1 on pool+ACT
                f4 = wp.tile([128, NCH, 4], F32, tag="f4", name=f"f4_{i}")
                nc.vector.tensor_tensor(f4[:, 0:128, :], f8[:, 0:128, 0:4],
                                        f8[:, 0:128, 4:8], op=OP.max)
                _relu_max(nc, wp, f4[:, 128:NCH, :], f8[:, 128:NCH, 0:4],
                          f8[:, 128:NCH, 4:8], 128 * 4, "a", i)
                f2 = wp.tile([128, NCH, 2], F32, tag="f2", name=f"f2_{i}")
                nc.vector.tensor_tensor(f2[:], f4[:, :, 0:2], f4[:, :, 2:4],
                                        op=OP.max)
                c16 = wp.tile([128, NCH], F32, tag="c16", name=f"c16_{i}")
                nc.vector.tensor_tensor(
                    c16[:], f2[:, :, 0:1].rearrange("p c w -> p (c w)"),
                    f2[:, :, 1:2].rearrange("p c w -> p (c w)"), op=OP.max)
                ids = sp.tile([128, NIDS], U16, tag="ids", name=f"ids_{i}")
                _rounds(nc, sp, c16[:], ids, "a", 3)
                nc.sync.dma_start(out=ids_d[ib * 128:(ib + 1) * 128, :], in_=ids[:])
    nc.compile()
    return nc


def _build_l2a(repeat=1):
    nc = bacc.Bacc("TRN2", target_bir_lowering=False, debug=False,
                   num_devices=NCORES)
    g_d = nc.dram_tensor("g", [NQ, 3 * W], F32, kind="ExternalInput").ap()
    q_d = nc.dram_tensor("q", [NQ, 3], F32, kind="ExternalInput").ap()
    loc_d = nc.dram_tensor("loc", [NQ, 16], U16, kind="ExternalOutput").ap()
    with tile.TileContext(nc) as tc:
        with (
            tc.tile_pool(name="tabs", bufs=1) as tabs,
            tc.tile_pool(name="work", bufs=3) as wp,
            tc.tile_pool(name="small", bufs=3) as sp,
        ):
            for i in range(repeat * NBLK):
                ib = i % NBLK
                sl = slice(ib * 128, (ib + 1) * 128)
                gt = wp.tile([128, 3 * W], F32, tag="gt", name=f"gt_{i}")
                qx = sp.tile([128, 3], F32, tag="qx", name=f"qx_{i}")
                nc.sync.dma_start(out=gt[:], in_=g_d[sl, :])
                nc.sync.dma_start(out=qx[:], in_=q_d[sl, :])
                nsq = wp.tile([128, 3, W], F32, tag="nsq", name=f"nsq_{i}")
                for c in range(3):
                    nc.scalar.activation(nsq[:, c, :], gt[:, c * W:(c + 1) * W],
                                         AF.Square, bias=qx[:, c:c + 1],
                                         scale=-1.0)
                t1 = wp.tile([128, W], F32, tag="t1", name=f"t1_{i}")
                nc.gpsimd.tensor_tensor(t1[:], nsq[:, 0, :], nsq[:, 1, :],
                                        op=OP.add)
                nd = wp.tile([128, W], F32, tag="nd", name=f"nd_{i}")
                nc.vector.scalar_tensor_tensor(
                    out=nd[:], in0=nsq[:, 2, :], scalar=-1.0, in1=t1[:],
                    op0=OP.mult, op1=OP.subtract)
                loc = sp.tile([128, 16], U16, tag="loc", name=f"loc_{i}")
                _rounds(nc, sp, nd[:], loc, "b", 2)
                nc.sync.dma_start(out=loc_d[sl, :], in_=loc[:])
    nc.compile()
    return nc


def _build_l2b(repeat=1):
    nc = bacc.Bacc("TRN2", target_bir_lowering=False, debug=False,
                   num_devices=NCORES)
    g12_d = nc.dram_tensor("g12", [12, NQ * 8], F32R, kind="ExternalInput").ap()
    w1_d = nc.dram_tensor("w1b", [12, 128], F32R, kind="ExternalInput").ap()
    w2_d = nc.dram_tensor("w2b", [128, 128], F32R, kind="ExternalInput").ap()
    w3_d = nc.dram_tensor("w3b", [128, 128], F32R, kind="ExternalInput").ap()
    eye_d = nc.dram_tensor("eye", [128, 128], F32, kind="ExternalInput").ap()
    out_d = nc.dram_tensor("out", [NQ, C], F32, kind="ExternalOutput").ap()
    with tile.TileContext(nc) as tc:
        with (
            tc.tile_pool(name="tabs", bufs=1) as tabs,
            tc.tile_pool(name="psum", bufs=2, space="PSUM") as pp,
            tc.tile_pool(name="psum3", bufs=2, space="PSUM") as pp3,
            tc.tile_pool(name="psumT", bufs=2, space="PSUM") as ppt,
            tc.tile_pool(name="work", bufs=4) as wp,
            tc.tile_pool(name="small", bufs=4) as sp,
        ):
            w1_sb = tabs.tile([12, 128], F32R)
            w2_sb = tabs.tile([128, 128], F32R)
            w3_sb = tabs.tile([128, 128], F32R)
            eye_sb = tabs.tile([128, 128], F32)
            g12_sb = tabs.tile([12, NQ * 8], F32R)
            nc.sync.dma_start(out=g12_sb[:, 0:1024], in_=g12_d[:, 0:1024])
            for sb, dd in ((w1_sb, w1_d), (w2_sb, w2_d), (w3_sb, w3_d),
                           (eye_sb, eye_d)):
                nc.sync.dma_start(out=sb[:], in_=dd[:])
            for dj in range(1, NBLK):
                nc.sync.dma_start(
                    out=g12_sb[:, dj * 1024:(dj + 1) * 1024],
                    in_=g12_d[:, dj * 1024:(dj + 1) * 1024])
            for i in range(repeat * NBLK):
                ib = i % NBLK
                mx = sp.tile([128, 128], F32, tag="mx", name=f"mx_{i}")
                for t in range(2):
                    cs = slice(ib * 1024 + t * 512, ib * 1024 + (t + 1) * 512)
                    ps1 = pp.tile([128, 512], F32, tag="ps1", name=f"ps1_{i}_{t}")
                    nc.tensor.matmul(ps1[:], w1_sb[:], g12_sb[:, cs],
                                     start=True, stop=True)
                    h1 = wp.tile([128, 512], F32R, tag="h1", name=f"h1_{i}_{t}")
                    nc.scalar.activation(h1[:], ps1[:], AF.Relu)
                    ps2 = pp.tile([128, 512], F32, tag="ps2", name=f"ps2_{i}_{t}")
                    nc.tensor.matmul(ps2[:], w2_sb[:], h1[:],
                                     start=True, stop=True)
                    h2 = wp.tile([128, 512], F32R, tag="h2", name=f"h2_{i}_{t}")
                    if t == 0:
                        # ReLU2 split: half on ACT, half on DVE
                        nc.scalar.activation(h2[:], ps2[:], AF.Relu)
                    else:
                        nc.vector.tensor_scalar(h2[:], ps2[:], 0.0, scalar2=None,
                                                op0=OP.max)
                    ps3 = pp3.tile([128, 512], F32, tag="ps3",
                                   name=f"ps3_{i}_{t}")
                    nc.tensor.matmul(ps3[:], w3_sb[:], h2[:],
                                     start=True, stop=True)
                    nc.vector.tensor_reduce(
                        mx[:, t * 64:(t + 1) * 64],
                        ps3[:].rearrange("m (q p) -> m q p", p=8),
                        axis=AX.X, op=OP.max)
                # transpose to query-major, then max the 2-point packing
                pst = ppt.tile([128, 128], F32, tag="pst", name=f"pst_{i}")
                nc.tensor.transpose(pst[:], mx[:], eye_sb[:])
                mxT = sp.tile([128, 128], F32, tag="mxT", name=f"mxT_{i}")
                nc.scalar.activation(mxT[:], pst[:], AF.Copy)
                fin = sp.tile([128, 64], F32, tag="fin", name=f"fin_{i}")
                nc.vector.tensor_tensor(fin[:], mxT[:, 0:64], mxT[:, 64:128],
                                        op=OP.max)
                nc.sync.dma_start(out=out_d[ib * 128:(ib + 1) * 128, :],
                                  in_=fin[:])
    nc.compile()
    return nc


class _Executor:
    """Cached multi-core PJRT executor for one prebuilt Bass program."""

    def __init__(self, nc):
        install_neuronx_cc_hook()
        self.nc = nc
        part_name = nc.partition_id_tensor.name if nc.partition_id_tensor else None
        in_names, out_names, out_avals, zero_outs = [], [], [], []
        for alloc in nc.m.functions[0].allocations:
            if not isinstance(alloc, mybir.MemoryLocationSet):
                continue
            name = alloc.memorylocations[0].name
            if alloc.kind == "ExternalInput":
                if name != part_name:
                    in_names.append(name)
            elif alloc.kind == "ExternalOutput":
                shape = tuple(alloc.tensor_shape)
                dtype = mybir.dt.np(alloc.dtype)
                out_names.append(name)
                out_avals.append(jax.core.ShapedArray(shape, dtype))
                zero_outs.append(_np.zeros(shape, dtype))
        self.in_names, self.out_names = in_names, out_names
        self.out_avals, self.zero_outs = out_avals, zero_outs
        n_params = len(in_names)
        all_names = in_names + out_names
        if part_name is not None:
            all_names = all_names + [part_name]

        def _body(*args):
            operands = list(args)
            if part_name is not None:
                operands.append(bass2jax.partition_id_tensor())
            return tuple(_bass_exec_p.bind(
                *operands,
                out_avals=tuple(out_avals),
                in_names=tuple(all_names),
                out_names=tuple(out_names),
                lowering_input_output_aliases=(),
                sim_require_finite=True,
                sim_require_nnan=True,
                nc=nc,
            ))

        devices = jax.devices()[:NCORES]
        mesh = Mesh(_np.asarray(devices), ("core",))
        n_outs = len(out_names)
        self._fn = jax.jit(
            shard_map(_body, mesh=mesh,
                      in_specs=(PartitionSpec("core"),) * (n_params + n_outs),
                      out_specs=(PartitionSpec("core"),) * n_outs,
                      check_rep=False),
            donate_argnums=tuple(range(n_params, n_params + n_outs)),
            keep_unused=True,
        )

    def prepare(self, in_maps):
        n = NCORES
        return [
            _np.concatenate([_np.asarray(in_maps[c][name]) for c in range(n)], axis=0)
            for name in self.in_names
        ]

    def run_prepared(self, concat_in):
        n = NCORES
        concat_zeros = [_np.zeros((n * z.shape[0], *z.shape[1:]), z.dtype)
                        for z in self.zero_outs]
        return self._fn(*concat_in, *concat_zeros)

    def __call__(self, in_maps):
        n = NCORES
        outs = self.run_prepared(self.prepare(in_maps))
        outs = [_np.asarray(o) for o in outs]
        return [
            {name: outs[i].reshape(n, *self.out_avals[i].shape)[c]
             for i, name in enumerate(self.out_names)}
            for c in range(n)
        ]


def _get_progs():
    if "l1" not in _progs:
        _progs["l1"] = _Executor(_build_l1())
        _progs["l2a"] = _Executor(_build_l2a())
        _progs["l2b"] = _Executor(_build_l2b())
    return _progs["l1"], _progs["l2a"], _progs["l2b"]


def _split3(x):
    """x (fp32) -> three fp32 arrays exactly representable in bf16 whose sum
    approximates x to ~2^-27 relative."""
    h = x.astype(BF).astype(np.float32)
    r = x - h
    m = r.astype(BF).astype(np.float32)
    l = (r - m).astype(BF).astype(np.float32)
    return h, m, l


# (lhsT split index, rhs split index) pairs for the 6 retained products
_PAIRS = [(0, 0), (0, 1), (1, 0), (0, 2), (1, 1), (2, 0)]


def _pack_splits(u_splits, x_splits, tail_rows):
    """Build the [21, n] row stack: 6 pairs x 3 coords + 3 tail rows."""
    rows = []
    for a, b in _PAIRS:
        src = u_splits[a] if x_splits is None else x_splits[b]
        for c in range(3):
            rows.append(src[:, c])
    rows.extend(tail_rows)
    return np.stack(rows).astype(BF)


def kernel(xyz, w1, w2, w3, k):
    xyz = np.asarray(xyz, dtype=np.float32)
    w1 = np.asarray(w1, dtype=np.float32)
    w2 = np.asarray(w2, dtype=np.float32)
    w3 = np.asarray(w3, dtype=np.float32)
    assert int(k) == K and xyz.shape == (B, N, 3)
    l1, l2a, l2b = _get_progs()
    cores = list(range(NCORES))

    # ---- L1: coarse chunk selection (triple-bf16-split exact matmul) ----
    xyzS_b = []
    for b in range(B):
        X = xyz[b]
        sq = (X * X).sum(axis=1, dtype=np.float32)
        xs = _split3(X)
        ss = _split3(sq)
        xyzS_b.append(_pack_splits(None, xs, [ss[0], ss[1], ss[2]]))
    in1 = []
    neg1 = -np.ones(NQ, np.float32)
    for c in cores:
        b, h = c // 2, c % 2
        Q = xyz[b, h * NQ:(h + 1) * NQ]
        us = _split3(2.0 * Q)
        qS = _pack_splits(us, None, [neg1, neg1, neg1])
        in1.append({"xyzS": xyzS_b[b], "qS": qS})
    r1 = l1(in1)

    # ---- host glue: superset gather (self chunk is always ids[0]) -------
    sup = []   # per-core (NQ, W) global candidate ids
    in2 = []
    offs = np.arange(CH)[None, None, :]
    for c in cores:
        b, h = c // 2, c % 2
        ids = r1[c]["ids"][:, :NSEL].astype(np.int64)      # (NQ, 20)
        gq = h * NQ + np.arange(NQ)                        # global query ids
        s = (ids[:, :, None] * CH + offs).reshape(NQ, W)
        sup.append(s)
        g = xyz[b][s].copy()                               # (NQ, W, 3)
        g[s == gq[:, None]] = 1.0e3                        # dummy the self point
        g3 = np.ascontiguousarray(g.transpose(0, 2, 1)).reshape(NQ, 3 * W)
        q3 = np.ascontiguousarray(xyz[b, h * NQ:(h + 1) * NQ])
        in2.append({"g": g3.astype(np.float32), "q": q3.astype(np.float32)})
    r2 = l2a(in2)

    # ---- host glue: final-16 gather ------------------------------------
    w1blkT = np.zeros((12, 128), np.float32)
    w1blkT[0:3, 0:64] = w1.T
    w1blkT[3:6, 64:128] = w1.T
    w1blkT[6:9, 0:64] = -w1.T
    w1blkT[9:12, 64:128] = -w1.T
    w2blkT = np.zeros((128, 128), np.float32)
    w2blkT[0:64, 0:64] = w2.T
    w2blkT[64:128, 64:128] = w2.T
    w3blkT = np.zeros((128, 128), np.float32)
    w3blkT[0:64, 0:64] = w3.T
    w3blkT[64:128, 64:128] = w3.T
    eye = np.eye(128, dtype=np.float32)
    in3 = []
    for c in cores:
        b, h = c // 2, c % 2
        loc = r2[c]["loc"].astype(np.int64)            # (NQ, 16)
        glob = np.take_along_axis(sup[c], loc, axis=1)           # (NQ, 16)
        g16 = xyz[b][glob]                                     # (NQ, 16, 3)
        gA, gB = g16[:, 0::2, :], g16[:, 1::2, :]
        g6 = np.concatenate([gA, gB], axis=2)                  # (NQ, 8, 6)
        g6 = np.ascontiguousarray(g6.transpose(2, 0, 1)).reshape(6, NQ * 8)
        q = xyz[b, h * NQ:(h + 1) * NQ]
        xq6 = np.repeat(np.concatenate([q, q], axis=1)[:, None, :], 8, axis=1)
        xq6 = np.ascontiguousarray(xq6.transpose(2, 0, 1)).reshape(6, NQ * 8)
        g12 = np.concatenate([g6, xq6], axis=0)                # (12, NQ*8)
        in3.append({"g12": g12.astype(np.float32), "w1b": w1blkT,
                    "w2b": w2blkT, "w3b": w3blkT, "eye": eye})
    r3 = l2b(in3)

    out = np.zeros((B, C, N), np.float32)
    for c in cores:
        b, h = c // 2, c % 2
        out[b, :, h * NQ:(h + 1) * NQ] = r3[c]["out"].T
    return out
